# revision 24
# baseline (speedup 1.0000x reference)
import sys

sys.path.insert(0, "/opt/trn_rl_repo")

from contextlib import ExitStack

import numpy as np

import concourse.bass as bass
import concourse.tile as tile
from concourse import bacc
from concourse import mybir

B, T, C = 4, 2048, 1024
NH, D = 16, 64
NCORES = 8
HPC = NH // NCORES          # heads per core
F = HPC * D                 # per-core feature slice (128)
P = 128
TCH = 512                   # token chunk for qkv phase
QCH = 512                   # q chunk in attention
KT = 128                    # k tile in attention
f32 = mybir.dt.float32
f32r = mybir.dt.float32r
AF = mybir.ActivationFunctionType


def build_nc(b=B, t=T, reps=1):
    """One-core SPMD program: this core owns 2 heads (its weight shards are inputs)."""
    bt = b * t
    n_cc = C // P               # contraction chunks for qkv (8)
    n_tch = t // TCH            # token chunks per batch
    n_jj = t // QCH             # q chunks per batch
    n_ktile_b = t // KT         # k tiles per batch

    nc = bacc.Bacc(None, target_bir_lowering=False)
    xT = nc.declare_dram_parameter("xT", [C, bt], f32r, isOutput=False)
    wq = nc.declare_dram_parameter("wq", [C, F], f32r, isOutput=False)
    wk = nc.declare_dram_parameter("wk", [C, F], f32r, isOutput=False)
    wv = nc.declare_dram_parameter("wv", [C, F], f32r, isOutput=False)
    wo = nc.declare_dram_parameter("wo", [F, C], f32r, isOutput=False)
    cos2 = nc.declare_dram_parameter("cos2", [P, t], f32, isOutput=False)
    sinS = nc.declare_dram_parameter("sinS", [P, t], f32, isOutput=False)
    mask0 = nc.declare_dram_parameter("mask0", [P, P], f32, isOutput=False)
    ident = nc.declare_dram_parameter("ident", [P, P], f32, isOutput=False)
    y = nc.declare_dram_parameter("y", [bt, C], f32, isOutput=True)

    xT_t = xT.rearrange("(o p) n -> p o n", p=P)   # [128, 8, bt]

    with tile.TileContext(nc) as tc, ExitStack() as ctx:
        consts = ctx.enter_context(tc.tile_pool(name="consts", bufs=1))
        xpool = ctx.enter_context(tc.tile_pool(name="xt", bufs=3))
        qkvp = ctx.enter_context(tc.tile_pool(name="qkv", bufs=2))
        rope = ctx.enter_context(tc.tile_pool(name="rope", bufs=2))
        ppool = ctx.enter_context(tc.tile_pool(name="pp", bufs=4))
        otp = ctx.enter_context(tc.tile_pool(name="otp", bufs=3))
        ystage = ctx.enter_context(tc.tile_pool(name="yst", bufs=4))
        ps_big = ctx.enter_context(tc.tile_pool(name="psbig", bufs=2, space="PSUM"))
        ps_o = ctx.enter_context(tc.tile_pool(name="pso", bufs=1, space="PSUM"))
        ps_mix = ctx.enter_context(tc.tile_pool(name="psmix", bufs=2, space="PSUM"))

        # resident constants (wq first: needed by the first matmul; bulk on scalar queue)
        wq_sb = consts.tile([P, n_cc, F], f32r)
        nc.sync.dma_start(wq_sb, wq.rearrange("(o p) f -> p o f", p=P))
        wk_sb = consts.tile([P, n_cc, F], f32r)
        nc.scalar.dma_start(wk_sb, wk.rearrange("(o p) f -> p o f", p=P))
        wv_sb = consts.tile([P, n_cc, F], f32r)
        nc.scalar.dma_start(wv_sb, wv.rearrange("(o p) f -> p o f", p=P))
        cos_sb = consts.tile([P, t], f32)
        nc.scalar.dma_start(cos_sb, cos2[:, :])
        sin_sb = consts.tile([P, t], f32)
        nc.scalar.dma_start(sin_sb, sinS[:, :])
        id_sb = consts.tile([P, P], f32)
        nc.scalar.dma_start(id_sb, ident[:, :])
        mask_sb = consts.tile([P, P], f32)
        nc.scalar.dma_start(mask_sb, mask0[:, :])
        wo_sb = consts.tile([P, C], f32r)
        nc.scalar.dma_start(wo_sb, wo[:, :])
        ones_col = consts.tile([P, n_ktile_b], f32)
        nc.vector.memset(ones_col, 1.0)

        copy_flip = [0]

        def psum_copy(dst, src):
            # alternate DVE / ACT so neither engine eats all psum->sbuf copies
            if copy_flip[0] % 3 != 2:
                nc.vector.tensor_copy(dst, src)
            else:
                nc.scalar.copy(dst, src)
            copy_flip[0] += 1

        def rope_apply(ps, dst, cos_sl, sin_sl):
            """dst = ps * cos + swap32(ps) * sin  (feature-major [128, TCH])."""
            raw = rope.tile([P, TCH], f32, tag="raw")
            nc.scalar.copy(raw, ps)
            rot = rope.tile([P, TCH], f32, tag="rot")
            H = D // 2  # 32
            for hh in range(2):
                b0 = hh * D
                nc.sync.dma_start(rot[b0 : b0 + H], raw[b0 + H : b0 + 2 * H])
                nc.sync.dma_start(rot[b0 + H : b0 + 2 * H], raw[b0 : b0 + H])
            nc.vector.tensor_mul(out=dst, in0=raw, in1=cos_sl)
            tmp = rope.tile([P, TCH], f32, tag="tmp")
            nc.vector.tensor_mul(out=tmp, in0=rot, in1=sin_sl)
            nc.vector.tensor_add(out=dst, in0=dst, in1=tmp)

        for rep in range(reps):
         for bi in range(b):
            t0 = bi * t

            # ---------------- phase 1: qkv + rope + v transpose ----------------
            q_sb = qkvp.tile([P, t], f32r, tag="q")
            k_sb = qkvp.tile([P, t], f32r, tag="k")
            v1_sb = qkvp.tile([P, n_ktile_b, 2 * D + 2], f32r, tag="v1")
            nc.vector.tensor_copy(v1_sb[:, :, D], ones_col)
            nc.vector.tensor_copy(v1_sb[:, :, 2 * D + 1], ones_col)

            for it in range(n_tch):
                c0 = it * TCH
                xt = xpool.tile([P, n_cc, TCH], f32r, tag="xt")
                dma_eng = nc.sync if (bi == 0 and it == 0) else nc.gpsimd
                for cc in range(n_cc):
                    dma_eng.dma_start(xt[:, cc], xT_t[:, cc, t0 + c0 : t0 + c0 + TCH])

                for wi, (which, w_sb) in enumerate((("q", wq_sb), ("k", wk_sb), ("v", wv_sb))):
                    ps = ps_mix.tile([P, TCH], f32, tag="py")
                    for cc in range(n_cc):
                        nc.tensor.matmul(
                            ps,
                            lhsT=w_sb[:, cc, :],
                            rhs=xt[:, cc, :],
                            start=(cc == 0),
                            stop=(cc == n_cc - 1),
                        )
                    if which == "q":
                        rope_apply(ps, q_sb[:, c0 : c0 + TCH],
                                   cos_sb[:, c0 : c0 + TCH], sin_sb[:, c0 : c0 + TCH])
                    elif which == "k":
                        rope_apply(ps, k_sb[:, c0 : c0 + TCH],
                                   cos_sb[:, c0 : c0 + TCH], sin_sb[:, c0 : c0 + TCH])
                    else:
                        vt_sb = rope.tile([P, TCH], f32, tag="vt")
                        nc.vector.tensor_copy(vt_sb, ps)
                        for s in range(TCH // P):
                            pst = ps_mix.tile([P, P], f32, tag="py")
                            nc.tensor.transpose(pst, vt_sb[:, s * P : (s + 1) * P], id_sb)
                            ti = it * (TCH // P) + s
                            psum_copy(v1_sb[:, ti, 0:D], pst[:, 0:D])
                            psum_copy(v1_sb[:, ti, D + 1 : 2 * D + 1], pst[:, D : 2 * D])

            # ---------------- phase 2: attention + out-proj ----------------
            for jj in range(n_jj):
                n_kt = 4 * jj + 4
                psO0 = ps_o.tile([P, QCH], f32, tag="o0")
                psO1 = ps_o.tile([P, QCH], f32, tag="o1")
                for i in range(n_kt):
                    lo = max(0, i * KT - jj * QCH)
                    qs = jj * QCH + lo
                    w = QCH - lo
                    psS = ps_big.tile([P, 2, QCH], f32, tag="mm")
                    kt_sl = slice(i * KT, (i + 1) * KT)
                    q_sl = slice(qs, jj * QCH + QCH)
                    nc.tensor.matmul(
                        psS[:, 0, lo:], lhsT=k_sb[0:D, kt_sl],
                        rhs=q_sb[0:D, q_sl], start=True, stop=True,
                    )
                    nc.tensor.matmul(
                        psS[:, 1, lo:], lhsT=k_sb[D : 2 * D, kt_sl],
                        rhs=q_sb[D : 2 * D, q_sl], start=True, stop=True,
                    )
                    Pp = ppool.tile([P, 2, QCH], f32r, tag="p0")
                    nc.scalar.activation(Pp[:, :, lo:], psS[:, :, lo:], AF.Exp, scale=0.125)
                    if i >= 4 * jj:  # diagonal tile: triangle mask both heads
                        nc.vector.tensor_mul(
                            Pp[:, :, lo : lo + P], Pp[:, :, lo : lo + P],
                            mask_sb[:, None, :].to_broadcast((P, 2, P)),
                        )
                    ki = i  # k-tile index within batch
                    nc.tensor.matmul(
                        psO0[0 : D + 1, lo:], lhsT=v1_sb[:, ki, 0 : D + 1],
                        rhs=Pp[:, 0, lo:], start=(i == 0), stop=(i == n_kt - 1),
                    )
                    nc.tensor.matmul(
                        psO1[0 : D + 1, lo:],
                        lhsT=v1_sb[:, ki, D + 1 : 2 * D + 2],
                        rhs=Pp[:, 1, lo:], start=(i == 0), stop=(i == n_kt - 1),
                    )

                # softmax normalization: OT[h] = O[h] / l[h]
                r0s = otp.tile([1, QCH], f32, tag="r0s")
                nc.vector.reciprocal(r0s, psO0[D : D + 1, :])
                r1s = otp.tile([1, QCH], f32, tag="r1s")
                nc.vector.reciprocal(r1s, psO1[D : D + 1, :])
                rb0 = otp.tile([D, QCH], f32, tag="rb0")
                nc.gpsimd.partition_broadcast(rb0, r0s)
                rb1 = otp.tile([D, QCH], f32, tag="rb1")
                nc.gpsimd.partition_broadcast(rb1, r1s)
                OT = otp.tile([P, QCH], f32r, tag="ot")
                nc.vector.tensor_mul(OT[0:D, :], psO0[0:D, :], rb0)
                nc.vector.tensor_mul(OT[D:P, :], psO1[0:D, :], rb1)

                # out-proj: y[tok, :] += ... (partial over this core's features)
                for qt in range(QCH // P):
                    for nh in range(C // 512):
                        psY = ps_mix.tile([P, 512], f32, tag="py")
                        nc.tensor.matmul(
                            psY, lhsT=OT[:, qt * P : (qt + 1) * P],
                            rhs=wo_sb[:, nh * 512 : (nh + 1) * 512],
                            start=True, stop=True,
                        )
                        ysb = ystage.tile([P, 512], f32, tag="ysb")
                        psum_copy(ysb, psY)
                        r0 = t0 + jj * QCH + qt * P
                        nc.scalar.dma_start(y[r0 : r0 + P, nh * 512 : (nh + 1) * 512], ysb)

    nc.compile()
    return nc


def host_consts(t=T):
    pos = np.arange(t, dtype=np.float32)[:, None]
    i = np.arange(0, D, 2, dtype=np.float32)[None, :]
    theta = pos / np.power(np.float32(10000.0), i / np.float32(D))
    cos = np.cos(theta).astype(np.float32)  # [t, 32]
    sin = np.sin(theta).astype(np.float32)
    cos2 = np.ascontiguousarray(np.tile(cos.T, (4, 1)))                                # [128, t]
    sinS = np.ascontiguousarray(np.tile(np.concatenate([-sin.T, sin.T], 0), (2, 1)))   # [128, t]
    r = np.arange(P)[:, None]
    c = np.arange(P)[None, :]
    mask0 = (r <= c).astype(np.float32)
    ident = np.eye(P, dtype=np.float32)
    return cos2, sinS, mask0, ident


def make_in_maps(x, w_qkv, w_out, b=B, t=T):
    x = np.asarray(x, np.float32)
    w_qkv = np.asarray(w_qkv, np.float32)
    w_out = np.asarray(w_out, np.float32)
    xT = np.ascontiguousarray(x.reshape(b * t, C).T)
    cos2, sinS, mask0, ident = host_consts(t)
    in_maps = []
    for c0 in range(NCORES):
        h0 = c0 * F
        in_maps.append({
            "xT": xT,
            "wq": np.ascontiguousarray(w_qkv[:, h0 : h0 + F]),
            "wk": np.ascontiguousarray(w_qkv[:, C + h0 : C + h0 + F]),
            "wv": np.ascontiguousarray(w_qkv[:, 2 * C + h0 : 2 * C + h0 + F]),
            "wo": np.ascontiguousarray(w_out[h0 : h0 + F, :]),
            "cos2": cos2, "sinS": sinS, "mask0": mask0, "ident": ident,
        })
    return in_maps


_REPL = {"xT", "cos2", "sinS", "mask0", "ident"}


class _Runner:
    """jit-once SPMD runner over jax.shard_map + the bass_exec custom call.

    Used instead of bass_utils.run_bass_kernel_spmd because the donation
    path in run_bass_via_pjrt hits NRT_EXEC_UNIT_UNRECOVERABLE at this
    problem size; passing non-donated zero output buffers (the kernel fully
    overwrites y) is stable. Replicating the shared inputs (xT, rope/mask
    constants) also cuts host->device traffic ~3x.
    """

    def __init__(self, nc, n_cores):
        import jax
        from jax.sharding import Mesh, PartitionSpec as PSpec
        from concourse import bass2jax

        bass2jax.install_neuronx_cc_hook()
        self.jax = jax
        self.n_cores = n_cores
        part_name = nc.partition_id_tensor.name if nc.partition_id_tensor else None
        in_names, out_names, out_avals, zero_outs = [], [], [], []
        for alloc in nc.m.functions[0].allocations:
            if not isinstance(alloc, mybir.MemoryLocationSet):
                continue
            name = alloc.memorylocations[0].name
            if alloc.kind == "ExternalInput":
                if name != part_name:
                    in_names.append(name)
            elif alloc.kind == "ExternalOutput":
                out_names.append(name)
                shape = tuple(alloc.tensor_shape)
                dtype = mybir.dt.np(alloc.dtype)
                out_avals.append(jax.core.ShapedArray(shape, dtype))
                zero_outs.append(np.zeros(shape, dtype))
        self.in_names, self.out_names = in_names, out_names
        self.out_avals, self.zero_outs = out_avals, zero_outs
        all_names = in_names + out_names + ([part_name] if part_name else [])

        def _body(*args):
            operands = list(args)
            if part_name is not None:
                operands.append(bass2jax.partition_id_tensor())
            outs = bass2jax._bass_exec_p.bind(
                *operands,
                out_avals=tuple(out_avals),
                in_names=tuple(all_names),
                out_names=tuple(out_names),
                lowering_input_output_aliases=(),
                sim_require_finite=False,
                sim_require_nnan=False,
                nc=nc,
            )
            return tuple(outs)

        try:
            from jax.experimental.shard_map import shard_map
        except ImportError:
            from jax.shard_map import shard_map
        devices = jax.devices()[:n_cores]
        self.mesh = Mesh(np.asarray(devices), ("core",))
        in_specs = tuple(
            PSpec() if nm in _REPL else PSpec("core") for nm in in_names
        ) + tuple(PSpec("core") for _ in out_names)
        out_specs = tuple(PSpec("core") for _ in out_names)
        self.fn = jax.jit(
            shard_map(_body, mesh=self.mesh, in_specs=in_specs,
                      out_specs=out_specs, check_rep=False),
            keep_unused=True,
        )

    def run(self, in_maps):
        args = []
        for nm in self.in_names:
            if nm in _REPL:
                args.append(np.asarray(in_maps[0][nm]))
            else:
                args.append(np.concatenate([np.asarray(m[nm]) for m in in_maps], axis=0))
        for z in self.zero_outs:
            args.append(np.zeros((self.n_cores * z.shape[0], *z.shape[1:]), z.dtype))
        outs = self.jax.block_until_ready(self.fn(*args))
        res = []
        for c in range(self.n_cores):
            res.append({
                nm: np.asarray(o).reshape(self.n_cores, *aval.shape)[c]
                for nm, aval, o in zip(self.out_names, self.out_avals, outs)
            })
        return res


_cache = {}


def kernel(x, w_qkv, w_out):
    if "runner" not in _cache:
        _cache["nc"] = build_nc()
        _cache["runner"] = _Runner(_cache["nc"], NCORES)
    in_maps = make_in_maps(x, w_qkv, w_out)
    results = _cache["runner"].run(in_maps)
    y = np.zeros((B * T, C), np.float32)
    for r in results:
        y += r["y"]
    return y.reshape(B, T, C)


# revision 31
# speedup vs baseline: 1.1147x; 1.1147x over previous
import sys

sys.path.insert(0, "/opt/trn_rl_repo")

from contextlib import ExitStack

import numpy as np

import concourse.bass as bass
import concourse.tile as tile
from concourse import bacc
from concourse import mybir

B, T, C = 4, 2048, 1024
NH, D = 16, 64
NCORES = 8
HPC = NH // NCORES          # heads per core
F = HPC * D                 # per-core feature slice (128)
P = 128
TCH = 512                   # token chunk for qkv phase
QCH = 512                   # q chunk in attention
KT = 128                    # k tile in attention
f32 = mybir.dt.float32
f32r = mybir.dt.float32r
AF = mybir.ActivationFunctionType


def build_nc(b=B, t=T, reps=1):
    """One-core SPMD program: this core owns 2 heads (its weight shards are inputs)."""
    bt = b * t
    n_cc = C // P               # contraction chunks for qkv (8)
    n_tch = t // TCH            # token chunks per batch
    n_jj = t // QCH             # q chunks per batch
    n_ktile_b = t // KT         # k tiles per batch

    nc = bacc.Bacc(None, target_bir_lowering=False)
    xT = nc.declare_dram_parameter("xT", [C, bt], f32r, isOutput=False)
    wq = nc.declare_dram_parameter("wq", [C, F], f32r, isOutput=False)
    wk = nc.declare_dram_parameter("wk", [C, F], f32r, isOutput=False)
    wv = nc.declare_dram_parameter("wv", [C, F], f32r, isOutput=False)
    wo = nc.declare_dram_parameter("wo", [F, C], f32r, isOutput=False)
    cos2 = nc.declare_dram_parameter("cos2", [P, t], f32, isOutput=False)
    sinS = nc.declare_dram_parameter("sinS", [P, t], f32, isOutput=False)
    mask0 = nc.declare_dram_parameter("mask0", [P, P], f32, isOutput=False)
    ident = nc.declare_dram_parameter("ident", [P, P], f32, isOutput=False)
    y = nc.declare_dram_parameter("y", [bt, C], f32, isOutput=True)

    xT_t = xT.rearrange("(o p) n -> p o n", p=P)   # [128, 8, bt]

    with tile.TileContext(nc) as tc, ExitStack() as ctx:
        consts = ctx.enter_context(tc.tile_pool(name="consts", bufs=1))
        xpool = ctx.enter_context(tc.tile_pool(name="xt", bufs=3))
        qkvp = ctx.enter_context(tc.tile_pool(name="qkv", bufs=2))
        rope = ctx.enter_context(tc.tile_pool(name="rope", bufs=2))
        ppool = ctx.enter_context(tc.tile_pool(name="pp", bufs=5))
        otp = ctx.enter_context(tc.tile_pool(name="otp", bufs=3))
        ystage = ctx.enter_context(tc.tile_pool(name="yst", bufs=4))
        ps_big = ctx.enter_context(tc.tile_pool(name="psbig", bufs=2, space="PSUM"))
        ps_o = ctx.enter_context(tc.tile_pool(name="pso", bufs=1, space="PSUM"))
        ps_mix = ctx.enter_context(tc.tile_pool(name="psmix", bufs=2, space="PSUM"))

        # resident constants (wq first: needed by the first matmul; bulk on scalar queue)
        wq_sb = consts.tile([P, n_cc, F], f32r)
        nc.sync.dma_start(wq_sb, wq.rearrange("(o p) f -> p o f", p=P))
        wk_sb = consts.tile([P, n_cc, F], f32r)
        nc.scalar.dma_start(wk_sb, wk.rearrange("(o p) f -> p o f", p=P))
        wv_sb = consts.tile([P, n_cc, F], f32r)
        nc.scalar.dma_start(wv_sb, wv.rearrange("(o p) f -> p o f", p=P))
        cos_sb = consts.tile([P, t], f32)
        nc.scalar.dma_start(cos_sb, cos2[:, :])
        sin_sb = consts.tile([P, t], f32)
        nc.scalar.dma_start(sin_sb, sinS[:, :])
        id_sb = consts.tile([P, P], f32)
        nc.scalar.dma_start(id_sb, ident[:, :])
        mask_sb = consts.tile([P, P], f32)
        nc.scalar.dma_start(mask_sb, mask0[:, :])
        wo_sb = consts.tile([P, C], f32r)
        nc.scalar.dma_start(wo_sb, wo[:, :])
        ones_col = consts.tile([P, n_ktile_b], f32)
        nc.vector.memset(ones_col, 1.0)

        copy_flip = [0]

        def psum_copy(dst, src):
            # alternate DVE / ACT so neither engine eats all psum->sbuf copies
            if copy_flip[0] % 3 != 2:
                nc.vector.tensor_copy(dst, src)
            else:
                nc.scalar.copy(dst, src)
            copy_flip[0] += 1

        def rope_apply(ps, dst, cos_sl, sin_sl):
            """dst = ps * cos + swap32(ps) * sin  (feature-major [128, TCH])."""
            raw = rope.tile([P, TCH], f32, tag="raw")
            nc.scalar.copy(raw, ps)
            rot = rope.tile([P, TCH], f32, tag="rot")
            H = D // 2  # 32
            for hh in range(2):
                b0 = hh * D
                nc.sync.dma_start(rot[b0 : b0 + H], raw[b0 + H : b0 + 2 * H])
                nc.sync.dma_start(rot[b0 + H : b0 + 2 * H], raw[b0 : b0 + H])
            nc.vector.tensor_mul(out=dst, in0=raw, in1=cos_sl)
            tmp = rope.tile([P, TCH], f32, tag="tmp")
            nc.vector.tensor_mul(out=tmp, in0=rot, in1=sin_sl)
            nc.vector.tensor_add(out=dst, in0=dst, in1=tmp)

        def alloc_batch():
            st = {
                "q": qkvp.tile([P, t], f32r, tag="q", name="q_sb"),
                "k": qkvp.tile([P, t], f32r, tag="k", name="k_sb"),
                "v1": qkvp.tile([P, n_ktile_b, 2 * D + 2], f32r, tag="v1", name="v1_sb"),
            }
            nc.vector.tensor_copy(st["v1"][:, :, D], ones_col)
            nc.vector.tensor_copy(st["v1"][:, :, 2 * D + 1], ones_col)
            return st

        def emit_qkv_chunk(bi, it, st, first=False):
            t0 = bi * t
            c0 = it * TCH
            q_sb, k_sb, v1_sb = st["q"], st["k"], st["v1"]
            xt = xpool.tile([P, n_cc, TCH], f32r, tag="xt")
            dma_eng = nc.sync if first else nc.gpsimd
            for cc in range(n_cc):
                dma_eng.dma_start(xt[:, cc], xT_t[:, cc, t0 + c0 : t0 + c0 + TCH])

            for which, w_sb in (("q", wq_sb), ("k", wk_sb), ("v", wv_sb)):
                ps = ps_mix.tile([P, TCH], f32, tag="py")
                for cc in range(n_cc):
                    nc.tensor.matmul(
                        ps,
                        lhsT=w_sb[:, cc, :],
                        rhs=xt[:, cc, :],
                        start=(cc == 0),
                        stop=(cc == n_cc - 1),
                    )
                if which == "q":
                    rope_apply(ps, q_sb[:, c0 : c0 + TCH],
                               cos_sb[:, c0 : c0 + TCH], sin_sb[:, c0 : c0 + TCH])
                elif which == "k":
                    rope_apply(ps, k_sb[:, c0 : c0 + TCH],
                               cos_sb[:, c0 : c0 + TCH], sin_sb[:, c0 : c0 + TCH])
                else:
                    vt_sb = rope.tile([P, TCH], f32, tag="vt")
                    nc.vector.tensor_copy(vt_sb, ps)
                    for s in range(TCH // P):
                        pst = ps_mix.tile([P, P], f32, tag="py")
                        nc.tensor.transpose(pst, vt_sb[:, s * P : (s + 1) * P], id_sb)
                        ti = it * (TCH // P) + s
                        psum_copy(v1_sb[:, ti, 0:D], pst[:, 0:D])
                        psum_copy(v1_sb[:, ti, D + 1 : 2 * D + 1], pst[:, D : 2 * D])

        def emit_attn_jj(bi, jj, st):
            t0 = bi * t
            q_sb, k_sb, v1_sb = st["q"], st["k"], st["v1"]
            if True:
                n_kt = 4 * jj + 4
                psO0 = ps_o.tile([P, QCH], f32, tag="o0")
                psO1 = ps_o.tile([P, QCH], f32, tag="o1")
                for i in range(n_kt):
                    lo = max(0, i * KT - jj * QCH)
                    qs = jj * QCH + lo
                    w = QCH - lo
                    psS = ps_big.tile([P, 2, QCH], f32, tag="mm")
                    kt_sl = slice(i * KT, (i + 1) * KT)
                    q_sl = slice(qs, jj * QCH + QCH)
                    nc.tensor.matmul(
                        psS[:, 0, lo:], lhsT=k_sb[0:D, kt_sl],
                        rhs=q_sb[0:D, q_sl], start=True, stop=True,
                    )
                    nc.tensor.matmul(
                        psS[:, 1, lo:], lhsT=k_sb[D : 2 * D, kt_sl],
                        rhs=q_sb[D : 2 * D, q_sl], start=True, stop=True,
                    )
                    Pp = ppool.tile([P, 2, QCH], f32r, tag="p0")
                    nc.scalar.activation(Pp[:, :, lo:], psS[:, :, lo:], AF.Exp, scale=0.125)
                    if i >= 4 * jj:  # diagonal tile: triangle mask both heads
                        nc.vector.tensor_mul(
                            Pp[:, :, lo : lo + P], Pp[:, :, lo : lo + P],
                            mask_sb[:, None, :].to_broadcast((P, 2, P)),
                        )
                    ki = i  # k-tile index within batch
                    nc.tensor.matmul(
                        psO0[0 : D + 1, lo:], lhsT=v1_sb[:, ki, 0 : D + 1],
                        rhs=Pp[:, 0, lo:], start=(i == 0), stop=(i == n_kt - 1),
                    )
                    nc.tensor.matmul(
                        psO1[0 : D + 1, lo:],
                        lhsT=v1_sb[:, ki, D + 1 : 2 * D + 2],
                        rhs=Pp[:, 1, lo:], start=(i == 0), stop=(i == n_kt - 1),
                    )

                # softmax normalization: OT[h] = O[h] / l[h]
                r0s = otp.tile([1, QCH], f32, tag="r0s")
                nc.vector.reciprocal(r0s, psO0[D : D + 1, :])
                r1s = otp.tile([1, QCH], f32, tag="r1s")
                nc.vector.reciprocal(r1s, psO1[D : D + 1, :])
                rb0 = otp.tile([D, QCH], f32, tag="rb0")
                nc.gpsimd.partition_broadcast(rb0, r0s)
                rb1 = otp.tile([D, QCH], f32, tag="rb1")
                nc.gpsimd.partition_broadcast(rb1, r1s)
                OT = otp.tile([P, QCH], f32r, tag="ot")
                nc.vector.tensor_mul(OT[0:D, :], psO0[0:D, :], rb0)
                nc.vector.tensor_mul(OT[D:P, :], psO1[0:D, :], rb1)

                # out-proj: y[tok, :] += ... (partial over this core's features)
                for qt in range(QCH // P):
                    for nh in range(C // 512):
                        psY = ps_mix.tile([P, 512], f32, tag="py")
                        nc.tensor.matmul(
                            psY, lhsT=OT[:, qt * P : (qt + 1) * P],
                            rhs=wo_sb[:, nh * 512 : (nh + 1) * 512],
                            start=True, stop=True,
                        )
                        ysb = ystage.tile([P, 512], f32, tag="ysb")
                        psum_copy(ysb, psY)
                        r0 = t0 + jj * QCH + qt * P
                        nc.scalar.dma_start(y[r0 : r0 + P, nh * 512 : (nh + 1) * 512], ysb)

        # software-pipelined emission: interleave qkv(b+1) chunks between
        # attention jj-blocks of batch b so the scheduler can fill the
        # ACT-bound attention stretches with qkv matmuls.
        for rep in range(reps):
            st_cur = alloc_batch()
            for it in range(n_tch):
                emit_qkv_chunk(0, it, st_cur, first=(rep == 0 and it == 0))
            for bi in range(b):
                st_next = alloc_batch() if bi + 1 < b else None
                for jj in range(n_jj):
                    emit_attn_jj(bi, jj, st_cur)
                    if st_next is not None:
                        emit_qkv_chunk(bi + 1, jj, st_next)
                st_cur = st_next

    nc.compile()
    return nc


def host_consts(t=T):
    pos = np.arange(t, dtype=np.float32)[:, None]
    i = np.arange(0, D, 2, dtype=np.float32)[None, :]
    theta = pos / np.power(np.float32(10000.0), i / np.float32(D))
    cos = np.cos(theta).astype(np.float32)  # [t, 32]
    sin = np.sin(theta).astype(np.float32)
    cos2 = np.ascontiguousarray(np.tile(cos.T, (4, 1)))                                # [128, t]
    sinS = np.ascontiguousarray(np.tile(np.concatenate([-sin.T, sin.T], 0), (2, 1)))   # [128, t]
    r = np.arange(P)[:, None]
    c = np.arange(P)[None, :]
    mask0 = (r <= c).astype(np.float32)
    ident = np.eye(P, dtype=np.float32)
    return cos2, sinS, mask0, ident


def make_in_maps(x, w_qkv, w_out, b=B, t=T):
    x = np.asarray(x, np.float32)
    w_qkv = np.asarray(w_qkv, np.float32)
    w_out = np.asarray(w_out, np.float32)
    xT = np.ascontiguousarray(x.reshape(b * t, C).T)
    cos2, sinS, mask0, ident = host_consts(t)
    in_maps = []
    for c0 in range(NCORES):
        h0 = c0 * F
        in_maps.append({
            "xT": xT,
            "wq": np.ascontiguousarray(w_qkv[:, h0 : h0 + F]),
            "wk": np.ascontiguousarray(w_qkv[:, C + h0 : C + h0 + F]),
            "wv": np.ascontiguousarray(w_qkv[:, 2 * C + h0 : 2 * C + h0 + F]),
            "wo": np.ascontiguousarray(w_out[h0 : h0 + F, :]),
            "cos2": cos2, "sinS": sinS, "mask0": mask0, "ident": ident,
        })
    return in_maps


_REPL = {"xT", "cos2", "sinS", "mask0", "ident"}


class _Runner:
    """jit-once SPMD runner over jax.shard_map + the bass_exec custom call.

    Used instead of bass_utils.run_bass_kernel_spmd because the donation
    path in run_bass_via_pjrt hits NRT_EXEC_UNIT_UNRECOVERABLE at this
    problem size; passing non-donated zero output buffers (the kernel fully
    overwrites y) is stable. Replicating the shared inputs (xT, rope/mask
    constants) also cuts host->device traffic ~3x.
    """

    def __init__(self, nc, n_cores):
        import jax
        from jax.sharding import Mesh, PartitionSpec as PSpec
        from concourse import bass2jax

        bass2jax.install_neuronx_cc_hook()
        self.jax = jax
        self.n_cores = n_cores
        part_name = nc.partition_id_tensor.name if nc.partition_id_tensor else None
        in_names, out_names, out_avals, zero_outs = [], [], [], []
        for alloc in nc.m.functions[0].allocations:
            if not isinstance(alloc, mybir.MemoryLocationSet):
                continue
            name = alloc.memorylocations[0].name
            if alloc.kind == "ExternalInput":
                if name != part_name:
                    in_names.append(name)
            elif alloc.kind == "ExternalOutput":
                out_names.append(name)
                shape = tuple(alloc.tensor_shape)
                dtype = mybir.dt.np(alloc.dtype)
                out_avals.append(jax.core.ShapedArray(shape, dtype))
                zero_outs.append(np.zeros(shape, dtype))
        self.in_names, self.out_names = in_names, out_names
        self.out_avals, self.zero_outs = out_avals, zero_outs
        all_names = in_names + out_names + ([part_name] if part_name else [])

        def _body(*args):
            operands = list(args)
            if part_name is not None:
                operands.append(bass2jax.partition_id_tensor())
            outs = bass2jax._bass_exec_p.bind(
                *operands,
                out_avals=tuple(out_avals),
                in_names=tuple(all_names),
                out_names=tuple(out_names),
                lowering_input_output_aliases=(),
                sim_require_finite=False,
                sim_require_nnan=False,
                nc=nc,
            )
            return tuple(outs)

        try:
            from jax.experimental.shard_map import shard_map
        except ImportError:
            from jax.shard_map import shard_map
        devices = jax.devices()[:n_cores]
        self.mesh = Mesh(np.asarray(devices), ("core",))
        in_specs = tuple(
            PSpec() if nm in _REPL else PSpec("core") for nm in in_names
        ) + tuple(PSpec("core") for _ in out_names)
        out_specs = tuple(PSpec("core") for _ in out_names)
        self.fn = jax.jit(
            shard_map(_body, mesh=self.mesh, in_specs=in_specs,
                      out_specs=out_specs, check_rep=False),
            keep_unused=True,
        )

    def run(self, in_maps):
        args = []
        for nm in self.in_names:
            if nm in _REPL:
                args.append(np.asarray(in_maps[0][nm]))
            else:
                args.append(np.concatenate([np.asarray(m[nm]) for m in in_maps], axis=0))
        for z in self.zero_outs:
            args.append(np.zeros((self.n_cores * z.shape[0], *z.shape[1:]), z.dtype))
        outs = self.jax.block_until_ready(self.fn(*args))
        res = []
        for c in range(self.n_cores):
            res.append({
                nm: np.asarray(o).reshape(self.n_cores, *aval.shape)[c]
                for nm, aval, o in zip(self.out_names, self.out_avals, outs)
            })
        return res


_cache = {}


def kernel(x, w_qkv, w_out):
    if "runner" not in _cache:
        _cache["nc"] = build_nc()
        _cache["runner"] = _Runner(_cache["nc"], NCORES)
    in_maps = make_in_maps(x, w_qkv, w_out)
    results = _cache["runner"].run(in_maps)
    y = np.zeros((B * T, C), np.float32)
    for r in results:
        y += r["y"]
    return y.reshape(B, T, C)
